# revision 25
# baseline (speedup 1.0000x reference)
"""Trainium2 Bass kernel for nn_Differentiate (B=32, C=3, L=E=512), 8-core data parallel.

Math (derived from the reference, per (b, c) pair):
  T0 = t0 @ Tw[c];  T1 = t1 @ Tw[c]
  Q  = (xlast @ Qw[c]) * T0 + Qb[c];  K = (u0 @ Kw[c]) * T1 + Kb[c]
  scores[l] = x0[l] . v,  v = Vw[c] @ (T1*Q*K)   (V_bias shifts scores by a
              constant -> dropped: softmax is shift invariant)
  p = softmax(scores)
  Ghat[l,:] = A0[l+1,:] - A0[l,:]  (l<511),  Ghat[511,:] = p
  dA[l,:] = Ghat[l,:] - A0[0,:]   (uniform, incl. row 511)
  dx = Ghat @ x0 - x0             (post-diff x1 minus x0; row 511 = p @ x0 - x0[511])
  du = w * ((g @ x0)/L - u0),   g = A0[511,:] - A0[0,:] + p  (telescoped col sums)
  w  = sum(T0*T1) / sqrt(sum(T0^2) + sum(T1^2))

Ghat^T is produced directly as matmul(lhsT=A0_natural, rhs=Z^T) with a constant
shift-difference matrix Z (f32r, 1 cycle/row); dA is formed in transposed space
by one tensor_scalar and transposed back through the PE.
"""

import numpy as np

B, C, L, E = 32, 3, 512, 512
NCORES = 8
BPC = B // NCORES  # samples per core
P = 128
NCH = L // P  # 4 chunks of 128

_cache = {}


def _patch_tile_drain():
    """This container's walrus can't encode >1 sync wait on one Drain
    (CoreV3 CTRL struct). Split the Tile tail-drain waits across several
    sequential drains on SP -- semantically identical."""
    import concourse.tile as tile
    import concourse.mybir as mybir
    from concourse.vector_clock import ScopedClock

    if getattr(tile.TileContext, "_drain_patched", False):
        return

    def _drain_and_barrier(self, tick_clock, wait_clock):
        nc = self.nc
        drain_inst = nc.sync.drain()
        wait_clock.add_sem_waits(
            drain_inst.ins, ScopedClock({None: tick_clock.global_clock})
        )
        si = drain_inst.ins.sync_info
        if si is not None and si.on_wait and len(si.on_wait) > 1:
            waits = list(si.on_wait)
            si.on_wait = [waits[0]]
            for w in waits[1:]:
                d = nc.sync.drain()
                d.ins.sync_info = mybir.SyncInfo(on_wait=[w], on_update=[])
        nc.all_engine_barrier()
        popped = nc._tile_sem_poison_stack.pop()
        assert popped is self._sem_poison
        nc.clear_and_free_semaphores(list(self.sems.allocated().values()))
        nc.all_engine_barrier()

    tile.TileContext._drain_and_barrier = _drain_and_barrier
    tile.TileContext._drain_patched = True


_WAIT_CAP = 1  # max on_wait entries walrus can encode per instruction


def _split_excess_waits(nc):
    """Move excess sem waits onto same-engine NoOps inserted just before the
    instruction (sequencers execute in order, so this is semantically
    identical); walrus here can't encode multiple waits per instruction."""
    import concourse.mybir as mybir

    n = [0]

    def fix_block(blk):
        out = []
        for inst in blk.instructions:
            si = inst.sync_info
            if si is not None and si.on_wait and len(si.on_wait) > _WAIT_CAP:
                waits = list(si.on_wait)
                si.on_wait = waits[-(_WAIT_CAP - 1):] if _WAIT_CAP > 1 else []
                head = waits[: len(waits) - len(si.on_wait)]
                for i in range(0, len(head), _WAIT_CAP):
                    n[0] += 1
                    nop = mybir.InstNoOp(
                        name=f"I-wsplit-{n[0]}",
                        engine=inst.engine,
                        sync_info=mybir.SyncInfo(
                            on_wait=head[i : i + _WAIT_CAP], on_update=[]
                        ),
                        bass_nofuse=True,
                    )
                    out.append(nop)
            out.append(inst)
        blk.instructions = out

    for blk in nc.m.functions[0].blocks:
        fix_block(blk)
    return n[0]


def _build_program(reps=1):
    import concourse.bass as bass
    import concourse.mybir as mybir
    import concourse.tile as tile
    from contextlib import ExitStack

    _patch_tile_drain()
    f32 = mybir.dt.float32
    f32r = mybir.dt.float32r
    AX = mybir.AxisListType.X
    OP = mybir.AluOpType
    AF = mybir.ActivationFunctionType

    nc = bass.Bass("TRN2")

    # ---- I/O ----
    A0_d = nc.dram_tensor("A0", (BPC, C, L, L), f32r, kind="ExternalInput")
    X0_d = nc.dram_tensor("x0", (BPC, C, L, E), f32r, kind="ExternalInput")
    t0T_d = nc.dram_tensor("t0T", (E, BPC), f32, kind="ExternalInput")
    t1T_d = nc.dram_tensor("t1T", (E, BPC), f32, kind="ExternalInput")
    u0T_d = nc.dram_tensor("u0T", (C, E, BPC), f32, kind="ExternalInput")
    xlT_d = nc.dram_tensor("xlastT", (C, E, BPC), f32, kind="ExternalInput")
    u0r_d = nc.dram_tensor("u0rows", (1, BPC * C, E), f32, kind="ExternalInput")
    a0r0_d = nc.dram_tensor("A0r0", (P, BPC * C * NCH), f32, kind="ExternalInput")
    TW_d = nc.dram_tensor("TW", (C, E, E), f32, kind="ExternalInput")
    QW_d = nc.dram_tensor("QW", (C, E, E), f32, kind="ExternalInput")
    KW_d = nc.dram_tensor("KW", (C, E, E), f32, kind="ExternalInput")
    VWT_d = nc.dram_tensor("VWT", (C, E, E), f32, kind="ExternalInput")
    QB_d = nc.dram_tensor("QB", (C, E), f32, kind="ExternalInput")
    KB_d = nc.dram_tensor("KB", (C, E), f32, kind="ExternalInput")
    ZT_d = nc.dram_tensor("ZT", (L, L), f32r, kind="ExternalInput")
    IDN_d = nc.dram_tensor("IDN", (P, P), f32, kind="ExternalInput")
    EB4_d = nc.dram_tensor("EB4", (BPC, BPC, P), f32r, kind="ExternalInput")

    dA_d = nc.dram_tensor("dA", (BPC, C, L, L), f32, kind="ExternalOutput")
    dX_d = nc.dram_tensor("dX", (BPC, C, L, E), f32, kind="ExternalOutput")
    dU_d = nc.dram_tensor("dU", (BPC, C, 1, E), f32, kind="ExternalOutput")

    with tile.TileContext(nc) as tc, ExitStack() as ctx:
        pre = ctx.enter_context(tc.tile_pool(name="pre", bufs=1))
        const = ctx.enter_context(tc.tile_pool(name="const", bufs=1))
        A0t0 = pre.tile([P, NCH, L], f32r)
        ZTs = const.tile([P, NCH, L], f32r)
        X0t0 = pre.tile([P, NCH, E], f32r)
        # chunked loads so the first Ghat matmuls start after ~0.5 MB
        for j in range(NCH):
            nc.sync.dma_start(
                A0t0[:, j, :],
                A0_d[0, 0, P * j : P * (j + 1), :].rearrange("(o p) m -> p o m", p=P))
            nc.sync.dma_start(
                ZTs[:, j, :],
                ZT_d[P * j : P * (j + 1), :].rearrange("(o p) l -> p o l", p=P))
        nc.sync.dma_start(X0t0, X0_d[0, 0, :, :].rearrange("(j p) m -> p j m", p=P))
        IDNs = const.tile([P, P], f32)
        nc.sync.dma_start(IDNs, IDN_d[:, :])
        EB4s = const.tile([BPC, BPC, P], f32r)
        nc.sync.dma_start(EB4s, EB4_d[:, :, :])
        t0Ts = const.tile([P, NCH, BPC], f32)
        nc.sync.dma_start(t0Ts, t0T_d[:, :].rearrange("(j p) b -> p j b", p=P))
        t1Ts = const.tile([P, NCH, BPC], f32)
        nc.sync.dma_start(t1Ts, t1T_d[:, :].rearrange("(j p) b -> p j b", p=P))
        u0Ts = const.tile([P, C, NCH, BPC], f32)
        nc.sync.dma_start(u0Ts, u0T_d[:, :, :].rearrange("c (j p) b -> p c j b", p=P))
        xlTs = const.tile([P, C, NCH, BPC], f32)
        nc.sync.dma_start(xlTs, xlT_d[:, :, :].rearrange("c (j p) b -> p c j b", p=P))
        u0rows = const.tile([1, BPC * C, E], f32)
        nc.sync.dma_start(u0rows, u0r_d[:, :, :])
        A0r0s = const.tile([P, BPC * C * NCH], f32)
        nc.sync.dma_start(A0r0s, a0r0_d[:, :])

        Vall = const.tile([BPC, C, E], f32r)     # per-(b,c) v row vectors
        wrow = const.tile([1, BPC * C], f32)    # w at column BPC*c+b
        w512row = const.tile([1, BPC * C], f32)  # w / L

        def emit_ghat(A0t, Ghs, A511, ppool):
            # Ghat^T (cols 0..510) + A0[511,:] column extraction; col 511
            # (the softmax row) is patched in later by the caller.
            for tm in range(NCH):
                gp = ppool.tile([P, L], f32, tag="bigp", bufs=6)
                for j in range(NCH):
                    nc.tensor.matmul(
                        gp, A0t[:, j, P * tm : P * (tm + 1)], ZTs[:, j, :],
                        start=(j == 0), stop=(j == NCH - 1))
                nc.scalar.copy(Ghs[:, tm, 0 : L - 1], gp[:, 0 : L - 1])
                nc.scalar.copy(A511[:, tm : tm + 1], gp[:, L - 1 : L])

        # bc0's Ghat runs before the setup matmuls so the PE has work while
        # the 13 MB of weights stream in.
        Ghs0 = pre.tile([P, NCH, L], f32r)
        A5110 = pre.tile([P, NCH], f32)
        with tc.tile_pool(name="prepsum", bufs=2, space="PSUM") as prepsum:
            emit_ghat(A0t0, Ghs0, A5110, prepsum)

        # ---------------- per-c setup ----------------
        with tc.tile_pool(name="wts", bufs=2) as wts, \
             tc.tile_pool(name="sps", bufs=1, space="PSUM") as sps, \
             tc.tile_pool(name="ssb", bufs=2) as ssb:
            for c in range(C):
                Tw = wts.tile([P, NCH, E], f32, tag="tw")
                nc.sync.dma_start(Tw, TW_d[c, :, :].rearrange("(j p) f -> p j f", p=P))
                Qw = wts.tile([P, NCH, E], f32, tag="qw")
                nc.sync.dma_start(Qw, QW_d[c, :, :].rearrange("(j p) f -> p j f", p=P))
                Kw = wts.tile([P, NCH, E], f32, tag="kw")
                nc.sync.dma_start(Kw, KW_d[c, :, :].rearrange("(j p) f -> p j f", p=P))

                T0p = sps.tile([BPC, E], f32, tag="t0p")
                T1p = sps.tile([BPC, E], f32, tag="t1p")
                Qp = sps.tile([BPC, E], f32, tag="qp")
                Kp = sps.tile([BPC, E], f32, tag="kp")
                for j in range(NCH):
                    st, sp = (j == 0), (j == NCH - 1)
                    nc.tensor.matmul(T0p, t0Ts[:, j, :], Tw[:, j, :], start=st, stop=sp)
                    nc.tensor.matmul(T1p, t1Ts[:, j, :], Tw[:, j, :], start=st, stop=sp)
                    nc.tensor.matmul(Qp, xlTs[:, c, j, :], Qw[:, j, :], start=st, stop=sp)
                    nc.tensor.matmul(Kp, u0Ts[:, c, j, :], Kw[:, j, :], start=st, stop=sp)

                T0s = ssb.tile([BPC, E], f32, tag="t0s")
                nc.scalar.copy(T0s, T0p)
                T1s = ssb.tile([BPC, E], f32, tag="t1s")
                nc.scalar.copy(T1s, T1p)

                qb4 = ssb.tile([BPC, E], f32, tag="qb4")
                nc.sync.dma_start(qb4, QB_d[c : c + 1, :].to_broadcast((BPC, E)))
                kb4 = ssb.tile([BPC, E], f32, tag="kb4")
                nc.sync.dma_start(kb4, KB_d[c : c + 1, :].to_broadcast((BPC, E)))

                Qm = ssb.tile([BPC, E], f32, tag="qm")
                nc.vector.tensor_tensor(out=Qm, in0=Qp, in1=T0s, op=OP.mult)
                Qs = ssb.tile([BPC, E], f32, tag="qs")
                nc.vector.tensor_tensor(out=Qs, in0=Qm, in1=qb4, op=OP.add)
                Km = ssb.tile([BPC, E], f32, tag="km")
                nc.vector.tensor_tensor(out=Km, in0=Kp, in1=T1s, op=OP.mult)
                Ks = ssb.tile([BPC, E], f32, tag="ks")
                nc.vector.tensor_tensor(out=Ks, in0=Km, in1=kb4, op=OP.add)

                QK = ssb.tile([BPC, E], f32, tag="qk")
                nc.vector.tensor_tensor(out=QK, in0=Qs, in1=Ks, op=OP.mult)
                w1c = ssb.tile([BPC, E], f32, tag="w1c")
                nc.vector.tensor_tensor(out=w1c, in0=QK, in1=T1s, op=OP.mult)

                # w = sum(T0*T1) / sqrt(sum(T0^2)+sum(T1^2)), per b
                prod = ssb.tile([BPC, E], f32, tag="wprod")
                num = ssb.tile([BPC, 1], f32, tag="wnum")
                nc.vector.scalar_tensor_tensor(
                    out=prod, in0=T0s, scalar=1.0, in1=T1s,
                    op0=OP.mult, op1=OP.mult, accum_out=num)
                sq0 = ssb.tile([BPC, E], f32, tag="wsq0")
                ra = ssb.tile([BPC, 1], f32, tag="wra")
                nc.vector.scalar_tensor_tensor(
                    out=sq0, in0=T0s, scalar=1.0, in1=T0s,
                    op0=OP.mult, op1=OP.mult, accum_out=ra)
                sq1 = ssb.tile([BPC, E], f32, tag="wsq1")
                rb = ssb.tile([BPC, 1], f32, tag="wrb")
                nc.vector.scalar_tensor_tensor(
                    out=sq1, in0=T1s, scalar=1.0, in1=T1s,
                    op0=OP.mult, op1=OP.mult, accum_out=rb)
                den2 = ssb.tile([BPC, 1], f32, tag="wden2")
                nc.vector.tensor_tensor(out=den2, in0=ra, in1=rb, op=OP.add)
                den = ssb.tile([BPC, 1], f32, tag="wden")
                nc.scalar.activation(den, den2, AF.Sqrt)
                rec = ssb.tile([BPC, 1], f32, tag="wrec")
                nc.vector.reciprocal(rec, den)
                w4 = ssb.tile([BPC, 1], f32, tag="w4")
                nc.vector.tensor_tensor(out=w4, in0=num, in1=rec, op=OP.mult)
                wT = sps.tile([1, BPC], f32, tag="smm")
                nc.tensor.transpose(wT, w4, IDNs[0:BPC, 0:BPC])
                nc.scalar.copy(wrow[0:1, BPC * c : BPC * (c + 1)], wT)

                # w1^T chunks, then all 4 b's v rows in one matmul group
                w1T = ssb.tile([P, NCH, BPC], f32, tag="w1T")
                for j in range(NCH):
                    w1Tp = sps.tile([P, BPC], f32, tag="smm")
                    nc.tensor.transpose(
                        w1Tp, w1c[:, P * j : P * (j + 1)], IDNs[0:BPC, 0:BPC])
                    nc.scalar.copy(w1T[:, j, :], w1Tp)

                VwT = wts.tile([P, NCH, E], f32, tag="vwt")
                nc.sync.dma_start(VwT, VWT_d[c, :, :].rearrange("(j p) f -> p j f", p=P))
                vr = sps.tile([BPC, E], f32, tag="smm")
                for j in range(NCH):
                    nc.tensor.matmul(vr, w1T[:, j, :], VwT[:, j, :],
                                     start=(j == 0), stop=(j == NCH - 1))
                nc.scalar.copy(Vall[:, c, :], vr)

            nc.vector.tensor_scalar_mul(w512row, wrow, 1.0 / float(L))

        # ---------------- main loop over (b, c) ----------------
        with tc.tile_pool(name="big", bufs=2) as big, \
             tc.tile_pool(name="small", bufs=2) as small, \
             tc.tile_pool(name="pbig", bufs=6, space="PSUM") as pbig, \
             tc.tile_pool(name="psmall", bufs=1, space="PSUM") as psmall:
            def heavy(st):
                """dd/dx, dA, sum-dd/du for a completed iteration (1-deep
                software pipeline: runs while the NEXT iteration's softmax
                is on the DVE/ACT)."""
                b, c, idx, wcol = st["b"], st["c"], st["idx"], st["wcol"]
                Ghs, X0t, g4 = st["Ghs"], st["X0t"], st["g4"]

                dXs = big.tile([P, NCH, E], f32, tag="dx", bufs=3)
                for tl in range(NCH):
                    ddp = pbig.tile([P, E], f32, tag="bigp")
                    for j in range(NCH):
                        nc.tensor.matmul(
                            ddp, Ghs[:, j, P * tl : P * (tl + 1)], X0t[:, j, :],
                            start=(j == 0), stop=(j == NCH - 1))
                    nc.vector.tensor_tensor(
                        out=dXs[:, tl, :], in0=ddp,
                        in1=X0t[:, tl, :].bitcast(f32), op=OP.subtract)
                nc.sync.dma_start(
                    dX_d[b, c, :, :].rearrange("(j p) m -> p j m", p=P), dXs)

                # dA^T = Ghat^T - A0[0,:] (cols), then transpose back
                dAT = big.tile([P, NCH, L], f32, tag="dat")
                for tm in range(NCH):
                    nc.vector.tensor_scalar(
                        out=dAT[:, tm, :], in0=Ghs[:, tm, :].bitcast(f32),
                        scalar1=A0r0s[:, idx * NCH + tm : idx * NCH + tm + 1],
                        scalar2=None, op0=OP.subtract)
                dAs = big.tile([P, NCH, L], f32, tag="da", bufs=2)
                for tl in range(NCH):
                    dap = pbig.tile([P, L], f32, tag="bigp")
                    for j in range(NCH):
                        nc.tensor.transpose(
                            dap[:, P * j : P * (j + 1)],
                            dAT[:, j, P * tl : P * (tl + 1)], IDNs)
                    nc.scalar.copy(dAs[:, tl, :], dap)
                nc.sync.dma_start(
                    dA_d[b, c, :, :].rearrange("(j p) m -> p j m", p=P), dAs)

                # du = w * ((g @ x0)/L - u0)
                sddp = psmall.tile([1, E], f32, tag="vb")
                for t in range(NCH):
                    nc.tensor.matmul(sddp, g4[:, t : t + 1], X0t[:, t, :],
                                     start=(t == 0), stop=(t == NCH - 1))
                u0w = small.tile([1, E], f32, tag="u0w")
                nc.vector.tensor_scalar_mul(
                    u0w, u0rows[0:1, idx, :], wrow[0:1, wcol : wcol + 1])
                durow = small.tile([1, E], f32, tag="du")
                nc.vector.scalar_tensor_tensor(
                    out=durow, in0=sddp,
                    scalar=w512row[0:1, wcol : wcol + 1], in1=u0w,
                    op0=OP.mult, op1=OP.subtract)
                nc.sync.dma_start(dU_d[b, c, :, :], durow)

            pairs = [(rep, b, c) for rep in range(reps)
                     for b in range(BPC) for c in range(C)]
            loaded = {0: (A0t0, X0t0)}
            pending = None
            for k, (rep, b, c) in enumerate(pairs):
                idx = b * C + c
                wcol = BPC * c + b

                A0t, X0t = loaded.pop(k)
                if k + 1 < len(pairs):
                    _, nb, ncc = pairs[k + 1]
                    A0n = big.tile([P, NCH, L], f32r, tag="a0", bufs=3)
                    nc.sync.dma_start(
                        A0n, A0_d[nb, ncc, :, :].rearrange("(j p) m -> p j m", p=P))
                    X0n = big.tile([P, NCH, E], f32r, tag="x0", bufs=3)
                    nc.sync.dma_start(
                        X0n, X0_d[nb, ncc, :, :].rearrange("(j p) m -> p j m", p=P))
                    loaded[k + 1] = (A0n, X0n)

                # scores (DVE) start immediately; Ghat matmuls fill the PE
                vb = psmall.tile([P, E], f32, tag="vb")
                nc.tensor.matmul(vb, EB4s[:, b, :], Vall[:, c, :],
                                 start=True, stop=True)
                if k == 0:
                    Ghs, A511 = Ghs0, A5110
                else:
                    Ghs = big.tile([P, NCH, L], f32r, tag="gh")
                    A511 = small.tile([P, NCH], f32, tag="a511")
                    emit_ghat(A0t, Ghs, A511, pbig)

                S4 = small.tile([P, NCH], f32, tag="s4")
                scr = small.tile([P, E], f32, tag="scr")
                for t in range(NCH):
                    nc.vector.scalar_tensor_tensor(
                        out=scr, in0=X0t[:, t, :].bitcast(f32), scalar=1.0,
                        in1=vb, op0=OP.mult, op1=OP.mult,
                        accum_out=S4[:, t : t + 1])
                srow = psmall.tile([1, L], f32, tag="srow")
                for t in range(NCH):
                    nc.tensor.transpose(
                        srow[0:1, P * t : P * (t + 1)], S4[:, t : t + 1], IDNs)
                negmx = small.tile([1, 1], f32, tag="negmx")
                nc.vector.reduce_max(negmx, srow, axis=AX, negate=True)
                erow = small.tile([1, L], f32, tag="erow")
                ssum = small.tile([1, 1], f32, tag="ssum")
                nc.scalar.activation(erow, srow, AF.Exp, bias=negmx, scale=1.0,
                                     accum_out=ssum)
                rcp = small.tile([1, 1], f32, tag="rcp")
                nc.vector.reciprocal(rcp, ssum)
                prow = small.tile([1, L], f32, tag="prow")
                nc.vector.tensor_scalar_mul(prow, erow, rcp)

                # previous iteration's big matmul block runs here, hiding
                # this iteration's softmax latency
                if pending is not None:
                    heavy(pending)

                pcolp = psmall.tile([P, NCH], f32, tag="srow")
                for t in range(NCH):
                    nc.tensor.transpose(
                        pcolp[:, t : t + 1], prow[0:1, P * t : P * (t + 1)],
                        IDNs[0:1, 0:1])
                P4s = small.tile([P, NCH], f32, tag="p4")
                nc.scalar.copy(P4s, pcolp)
                for tm in range(NCH):
                    nc.scalar.copy(Ghs[:, tm, L - 1 : L], P4s[:, tm : tm + 1])

                # g = A0[511,:] - A0[0,:] + p  (as columns per m-chunk)
                tmp4 = small.tile([P, NCH], f32, tag="tmp4")
                nc.vector.tensor_tensor(
                    out=tmp4, in0=A511,
                    in1=A0r0s[:, idx * NCH : (idx + 1) * NCH], op=OP.subtract)
                g4 = small.tile([P, NCH], f32r, tag="g4")
                nc.vector.tensor_tensor(out=g4, in0=tmp4, in1=P4s, op=OP.add)

                pending = dict(b=b, c=c, idx=idx, wcol=wcol,
                               Ghs=Ghs, X0t=X0t, g4=g4)
            heavy(pending)

    nsplit = _split_excess_waits(nc)
    if nsplit:
        print(f"[kernel] split {nsplit} over-cap sem-wait groups onto NoOps")
    return nc


def _constants():
    ZT = np.zeros((L, L), np.float32)
    li = np.arange(L - 1)
    ZT[li + 1, li] = 1.0
    ZT[li, li] = -1.0
    ZT[L - 1, L - 1] = 1.0  # extracts A0[511,:] into Ghat psum col 511

    IDN = np.eye(P, dtype=np.float32)
    EB4 = np.zeros((BPC, BPC, P), np.float32)
    for b in range(BPC):
        EB4[b, b, :] = 1.0
    return ZT, IDN, EB4


def kernel(A0, x0, u0, t0, t1,
           Q_weight, Q_bias, V_weight, V_bias,
           K_weight, K_bias, T_weight, T_bias):
    from concourse.bass_utils import run_bass_kernel_spmd

    A0 = np.asarray(A0, np.float32)
    x0 = np.asarray(x0, np.float32)
    u0 = np.asarray(u0, np.float32)
    t0 = np.asarray(t0, np.float32)
    t1 = np.asarray(t1, np.float32)

    if "nc" not in _cache:
        _cache["nc"] = _build_program()
    nc = _cache["nc"]

    ZT, IDN, EB4 = _constants()
    shared = {
        "TW": np.asarray(T_weight, np.float32),
        "QW": np.asarray(Q_weight, np.float32),
        "KW": np.asarray(K_weight, np.float32),
        "VWT": np.ascontiguousarray(np.asarray(V_weight, np.float32).transpose(0, 2, 1)),
        "QB": np.asarray(Q_bias, np.float32),
        "KB": np.asarray(K_bias, np.float32),
        "ZT": ZT, "IDN": IDN, "EB4": EB4,
    }

    in_maps = []
    for i in range(NCORES):
        sl = slice(BPC * i, BPC * (i + 1))
        A0s, x0s, u0s = A0[sl], x0[sl], u0[sl]
        m = dict(shared)
        m["A0"] = A0s
        m["x0"] = x0s
        m["t0T"] = np.ascontiguousarray(t0[sl].T)
        m["t1T"] = np.ascontiguousarray(t1[sl].T)
        m["u0T"] = np.ascontiguousarray(u0s.transpose(1, 2, 0))
        m["xlastT"] = np.ascontiguousarray(x0s[:, :, L - 1, :].transpose(1, 2, 0))
        m["u0rows"] = np.ascontiguousarray(u0s.reshape(1, BPC * C, E))
        # A0r0s[p, ((b*C+c)*NCH)+t] = A0[b, c, 0, 128*t+p]
        m["A0r0"] = np.ascontiguousarray(
            A0s[:, :, 0, :].reshape(BPC, C, NCH, P).transpose(3, 0, 1, 2)
            .reshape(P, BPC * C * NCH))
        in_maps.append(m)

    try:
        res = run_bass_kernel_spmd(nc, in_maps, core_ids=list(range(NCORES)))
    except Exception:
        # transient NRT_EXEC_UNIT_UNRECOVERABLE has been observed on the first
        # execution after a fresh load; a retry on the cleared device works
        import time as _time
        _time.sleep(2.0)
        res = run_bass_kernel_spmd(nc, in_maps, core_ids=list(range(NCORES)))

    dA = np.concatenate([r["dA"] for r in res.results], axis=0)
    dX = np.concatenate([r["dX"] for r in res.results], axis=0)
    dU = np.concatenate([r["dU"] for r in res.results], axis=0).reshape(B, C, E)
    return dA, dX, dU


# revision 29
# speedup vs baseline: 1.0203x; 1.0203x over previous
"""Trainium2 Bass kernel for nn_Differentiate (B=32, C=3, L=E=512), 8-core data parallel.

Math (derived from the reference, per (b, c) pair):
  T0 = t0 @ Tw[c];  T1 = t1 @ Tw[c]
  Q  = (xlast @ Qw[c]) * T0 + Qb[c];  K = (u0 @ Kw[c]) * T1 + Kb[c]
  scores[l] = x0[l] . v,  v = Vw[c] @ (T1*Q*K)   (V_bias shifts scores by a
              constant -> dropped: softmax is shift invariant)
  p = softmax(scores)
  Ghat[l,:] = A0[l+1,:] - A0[l,:]  (l<511),  Ghat[511,:] = p
  dA[l,:] = Ghat[l,:] - A0[0,:]   (uniform, incl. row 511)
  dx = Ghat @ x0 - x0             (post-diff x1 minus x0; row 511 = p @ x0 - x0[511])
  du = w * ((g @ x0)/L - u0),   g = A0[511,:] - A0[0,:] + p  (telescoped col sums)
  w  = sum(T0*T1) / sqrt(sum(T0^2) + sum(T1^2))

Ghat^T is produced directly as matmul(lhsT=A0_natural, rhs=Z^T) with a constant
shift-difference matrix Z (f32r, 1 cycle/row); dA is formed in transposed space
by one tensor_scalar and transposed back through the PE.
"""

import numpy as np

B, C, L, E = 32, 3, 512, 512
NCORES = 8
BPC = B // NCORES  # samples per core
P = 128
NCH = L // P  # 4 chunks of 128

_cache = {}


def _patch_tile_drain():
    """This container's walrus can't encode >1 sync wait on one Drain
    (CoreV3 CTRL struct). Split the Tile tail-drain waits across several
    sequential drains on SP -- semantically identical."""
    import concourse.tile as tile
    import concourse.mybir as mybir
    from concourse.vector_clock import ScopedClock

    if getattr(tile.TileContext, "_drain_patched", False):
        return

    def _drain_and_barrier(self, tick_clock, wait_clock):
        nc = self.nc
        drain_inst = nc.sync.drain()
        wait_clock.add_sem_waits(
            drain_inst.ins, ScopedClock({None: tick_clock.global_clock})
        )
        si = drain_inst.ins.sync_info
        if si is not None and si.on_wait and len(si.on_wait) > 1:
            waits = list(si.on_wait)
            si.on_wait = [waits[0]]
            for w in waits[1:]:
                d = nc.sync.drain()
                d.ins.sync_info = mybir.SyncInfo(on_wait=[w], on_update=[])
        nc.all_engine_barrier()
        popped = nc._tile_sem_poison_stack.pop()
        assert popped is self._sem_poison
        nc.clear_and_free_semaphores(list(self.sems.allocated().values()))
        nc.all_engine_barrier()

    tile.TileContext._drain_and_barrier = _drain_and_barrier
    tile.TileContext._drain_patched = True


_WAIT_CAP = 1  # max on_wait entries walrus can encode per instruction


def _split_excess_waits(nc):
    """Move excess sem waits onto same-engine NoOps inserted just before the
    instruction (sequencers execute in order, so this is semantically
    identical); walrus here can't encode multiple waits per instruction."""
    import concourse.mybir as mybir

    n = [0]

    def fix_block(blk):
        out = []
        for inst in blk.instructions:
            si = inst.sync_info
            if si is not None and si.on_wait and len(si.on_wait) > _WAIT_CAP:
                waits = list(si.on_wait)
                si.on_wait = waits[-(_WAIT_CAP - 1):] if _WAIT_CAP > 1 else []
                head = waits[: len(waits) - len(si.on_wait)]
                for i in range(0, len(head), _WAIT_CAP):
                    n[0] += 1
                    nop = mybir.InstNoOp(
                        name=f"I-wsplit-{n[0]}",
                        engine=inst.engine,
                        sync_info=mybir.SyncInfo(
                            on_wait=head[i : i + _WAIT_CAP], on_update=[]
                        ),
                        bass_nofuse=True,
                    )
                    out.append(nop)
            out.append(inst)
        blk.instructions = out

    for blk in nc.m.functions[0].blocks:
        fix_block(blk)
    return n[0]


def _build_program(reps=1):
    import concourse.bass as bass
    import concourse.mybir as mybir
    import concourse.tile as tile
    from contextlib import ExitStack

    _patch_tile_drain()
    f32 = mybir.dt.float32
    f32r = mybir.dt.float32r
    AX = mybir.AxisListType.X
    OP = mybir.AluOpType
    AF = mybir.ActivationFunctionType

    nc = bass.Bass("TRN2")

    # ---- I/O ----
    A0_d = nc.dram_tensor("A0", (BPC, C, L, L), f32r, kind="ExternalInput")
    X0_d = nc.dram_tensor("x0", (BPC, C, L, E), f32r, kind="ExternalInput")
    t0T_d = nc.dram_tensor("t0T", (E, BPC), f32, kind="ExternalInput")
    t1T_d = nc.dram_tensor("t1T", (E, BPC), f32, kind="ExternalInput")
    u0T_d = nc.dram_tensor("u0T", (C, E, BPC), f32, kind="ExternalInput")
    xlT_d = nc.dram_tensor("xlastT", (C, E, BPC), f32, kind="ExternalInput")
    u0r_d = nc.dram_tensor("u0rows", (1, BPC * C, E), f32, kind="ExternalInput")
    a0r0_d = nc.dram_tensor("A0r0", (P, BPC * C * NCH), f32, kind="ExternalInput")
    TW_d = nc.dram_tensor("TW", (C, E, E), f32, kind="ExternalInput")
    QW_d = nc.dram_tensor("QW", (C, E, E), f32, kind="ExternalInput")
    KW_d = nc.dram_tensor("KW", (C, E, E), f32, kind="ExternalInput")
    VWT_d = nc.dram_tensor("VWT", (C, E, E), f32, kind="ExternalInput")
    QB_d = nc.dram_tensor("QB", (C, E), f32, kind="ExternalInput")
    KB_d = nc.dram_tensor("KB", (C, E), f32, kind="ExternalInput")
    ZT_d = nc.dram_tensor("ZT", (L, L), f32r, kind="ExternalInput")
    IDN_d = nc.dram_tensor("IDN", (P, P), f32, kind="ExternalInput")
    EB4_d = nc.dram_tensor("EB4", (BPC, BPC, P), f32r, kind="ExternalInput")

    dA_d = nc.dram_tensor("dA", (BPC, C, L, L), f32, kind="ExternalOutput")
    dX_d = nc.dram_tensor("dX", (BPC, C, L, E), f32, kind="ExternalOutput")
    dU_d = nc.dram_tensor("dU", (BPC, C, 1, E), f32, kind="ExternalOutput")

    with tile.TileContext(nc) as tc, ExitStack() as ctx:
        pre = ctx.enter_context(tc.tile_pool(name="pre", bufs=1))
        const = ctx.enter_context(tc.tile_pool(name="const", bufs=1))
        A0t0 = pre.tile([P, NCH, L], f32r)
        ZTs = const.tile([P, NCH, L], f32r)
        X0t0 = pre.tile([P, NCH, E], f32r)
        # chunked loads so the first Ghat matmuls start after ~0.5 MB
        for j in range(NCH):
            nc.sync.dma_start(
                A0t0[:, j, :],
                A0_d[0, 0, P * j : P * (j + 1), :].rearrange("(o p) m -> p o m", p=P))
            nc.sync.dma_start(
                ZTs[:, j, :],
                ZT_d[P * j : P * (j + 1), :].rearrange("(o p) l -> p o l", p=P))
        nc.sync.dma_start(X0t0, X0_d[0, 0, :, :].rearrange("(j p) m -> p j m", p=P))
        IDNs = const.tile([P, P], f32)
        nc.sync.dma_start(IDNs, IDN_d[:, :])
        EB4s = const.tile([BPC, BPC, P], f32r)
        nc.sync.dma_start(EB4s, EB4_d[:, :, :])
        t0Ts = const.tile([P, NCH, BPC], f32)
        nc.sync.dma_start(t0Ts, t0T_d[:, :].rearrange("(j p) b -> p j b", p=P))
        t1Ts = const.tile([P, NCH, BPC], f32)
        nc.sync.dma_start(t1Ts, t1T_d[:, :].rearrange("(j p) b -> p j b", p=P))
        u0Ts = const.tile([P, C, NCH, BPC], f32)
        nc.sync.dma_start(u0Ts, u0T_d[:, :, :].rearrange("c (j p) b -> p c j b", p=P))
        xlTs = const.tile([P, C, NCH, BPC], f32)
        nc.sync.dma_start(xlTs, xlT_d[:, :, :].rearrange("c (j p) b -> p c j b", p=P))
        u0rows = const.tile([1, BPC * C, E], f32)
        nc.sync.dma_start(u0rows, u0r_d[:, :, :])
        A0r0s = const.tile([P, BPC * C * NCH], f32)
        nc.sync.dma_start(A0r0s, a0r0_d[:, :])

        Vall = const.tile([BPC, C, E], f32r)     # per-(b,c) v row vectors
        wrow = const.tile([1, BPC * C], f32)    # w at column BPC*c+b
        w512row = const.tile([1, BPC * C], f32)  # w / L

        def emit_ghat(A0t, Ghs, A511, ppool):
            # Ghat^T (cols 0..510) + A0[511,:] column extraction; col 511
            # (the softmax row) is patched in later by the caller.
            for tm in range(NCH):
                gp = ppool.tile([P, L], f32, tag="bigp", bufs=6)
                for j in range(NCH):
                    nc.tensor.matmul(
                        gp, A0t[:, j, P * tm : P * (tm + 1)], ZTs[:, j, :],
                        start=(j == 0), stop=(j == NCH - 1))
                nc.scalar.copy(Ghs[:, tm, 0 : L - 1], gp[:, 0 : L - 1])
                nc.scalar.copy(A511[:, tm : tm + 1], gp[:, L - 1 : L])

        # bc0's Ghat runs before the setup matmuls so the PE has work while
        # the 13 MB of weights stream in.
        Ghs0 = pre.tile([P, NCH, L], f32r)
        A5110 = pre.tile([P, NCH], f32)
        with tc.tile_pool(name="prepsum", bufs=2, space="PSUM") as prepsum:
            emit_ghat(A0t0, Ghs0, A5110, prepsum)

        # ---------------- per-c setup ----------------
        with tc.tile_pool(name="wts", bufs=2) as wts, \
             tc.tile_pool(name="sps", bufs=1, space="PSUM") as sps, \
             tc.tile_pool(name="ssb", bufs=2) as ssb:
            for c in range(C):
                Tw = wts.tile([P, NCH, E], f32, tag="tw")
                Qw = wts.tile([P, NCH, E], f32, tag="qw")
                Kw = wts.tile([P, NCH, E], f32, tag="kw")
                VwT = wts.tile([P, NCH, E], f32, tag="vwt")
                nc.sync.dma_start(Tw, TW_d[c, :, :].rearrange("(j p) f -> p j f", p=P))
                nc.sync.dma_start(Qw, QW_d[c, :, :].rearrange("(j p) f -> p j f", p=P))
                nc.sync.dma_start(Kw, KW_d[c, :, :].rearrange("(j p) f -> p j f", p=P))
                nc.sync.dma_start(VwT, VWT_d[c, :, :].rearrange("(j p) f -> p j f", p=P))

                T0p = sps.tile([BPC, E], f32, tag="t0p")
                T1p = sps.tile([BPC, E], f32, tag="t1p")
                Qp = sps.tile([BPC, E], f32, tag="qp")
                Kp = sps.tile([BPC, E], f32, tag="kp")
                for j in range(NCH):
                    st, sp = (j == 0), (j == NCH - 1)
                    nc.tensor.matmul(T0p, t0Ts[:, j, :], Tw[:, j, :], start=st, stop=sp)
                    nc.tensor.matmul(T1p, t1Ts[:, j, :], Tw[:, j, :], start=st, stop=sp)
                    nc.tensor.matmul(Qp, xlTs[:, c, j, :], Qw[:, j, :], start=st, stop=sp)
                    nc.tensor.matmul(Kp, u0Ts[:, c, j, :], Kw[:, j, :], start=st, stop=sp)

                T0s = ssb.tile([BPC, E], f32, tag="t0s")
                nc.scalar.copy(T0s, T0p)
                T1s = ssb.tile([BPC, E], f32, tag="t1s")
                nc.scalar.copy(T1s, T1p)

                qb4 = ssb.tile([BPC, E], f32, tag="qb4")
                nc.sync.dma_start(qb4, QB_d[c : c + 1, :].to_broadcast((BPC, E)))
                kb4 = ssb.tile([BPC, E], f32, tag="kb4")
                nc.sync.dma_start(kb4, KB_d[c : c + 1, :].to_broadcast((BPC, E)))

                Qm = ssb.tile([BPC, E], f32, tag="qm")
                nc.vector.tensor_tensor(out=Qm, in0=Qp, in1=T0s, op=OP.mult)
                Qs = ssb.tile([BPC, E], f32, tag="qs")
                nc.vector.tensor_tensor(out=Qs, in0=Qm, in1=qb4, op=OP.add)
                Km = ssb.tile([BPC, E], f32, tag="km")
                nc.vector.tensor_tensor(out=Km, in0=Kp, in1=T1s, op=OP.mult)
                Ks = ssb.tile([BPC, E], f32, tag="ks")
                nc.vector.tensor_tensor(out=Ks, in0=Km, in1=kb4, op=OP.add)

                QK = ssb.tile([BPC, E], f32, tag="qk")
                nc.vector.tensor_tensor(out=QK, in0=Qs, in1=Ks, op=OP.mult)
                w1c = ssb.tile([BPC, E], f32, tag="w1c")
                nc.vector.tensor_tensor(out=w1c, in0=QK, in1=T1s, op=OP.mult)

                # w = sum(T0*T1) / sqrt(sum(T0^2)+sum(T1^2)), per b
                prod = ssb.tile([BPC, E], f32, tag="wprod")
                num = ssb.tile([BPC, 1], f32, tag="wnum")
                nc.vector.scalar_tensor_tensor(
                    out=prod, in0=T0s, scalar=1.0, in1=T1s,
                    op0=OP.mult, op1=OP.mult, accum_out=num)
                sq0 = ssb.tile([BPC, E], f32, tag="wsq0")
                ra = ssb.tile([BPC, 1], f32, tag="wra")
                nc.vector.scalar_tensor_tensor(
                    out=sq0, in0=T0s, scalar=1.0, in1=T0s,
                    op0=OP.mult, op1=OP.mult, accum_out=ra)
                sq1 = ssb.tile([BPC, E], f32, tag="wsq1")
                rb = ssb.tile([BPC, 1], f32, tag="wrb")
                nc.vector.scalar_tensor_tensor(
                    out=sq1, in0=T1s, scalar=1.0, in1=T1s,
                    op0=OP.mult, op1=OP.mult, accum_out=rb)
                den2 = ssb.tile([BPC, 1], f32, tag="wden2")
                nc.vector.tensor_tensor(out=den2, in0=ra, in1=rb, op=OP.add)
                den = ssb.tile([BPC, 1], f32, tag="wden")
                nc.scalar.activation(den, den2, AF.Sqrt)
                rec = ssb.tile([BPC, 1], f32, tag="wrec")
                nc.vector.reciprocal(rec, den)
                w4 = ssb.tile([BPC, 1], f32, tag="w4")
                nc.vector.tensor_tensor(out=w4, in0=num, in1=rec, op=OP.mult)
                wT = sps.tile([1, BPC], f32, tag="smm")
                nc.tensor.transpose(wT, w4, IDNs[0:BPC, 0:BPC])
                nc.scalar.copy(wrow[0:1, BPC * c : BPC * (c + 1)], wT)

                # w1^T chunks, then all 4 b's v rows in one matmul group
                w1T = ssb.tile([P, NCH, BPC], f32, tag="w1T")
                for j in range(NCH):
                    w1Tp = sps.tile([P, BPC], f32, tag="smm")
                    nc.tensor.transpose(
                        w1Tp, w1c[:, P * j : P * (j + 1)], IDNs[0:BPC, 0:BPC])
                    nc.scalar.copy(w1T[:, j, :], w1Tp)

                vr = sps.tile([BPC, E], f32, tag="smm")
                for j in range(NCH):
                    nc.tensor.matmul(vr, w1T[:, j, :], VwT[:, j, :],
                                     start=(j == 0), stop=(j == NCH - 1))
                nc.scalar.copy(Vall[:, c, :], vr)

            nc.vector.tensor_scalar_mul(w512row, wrow, 1.0 / float(L))

        # ---------------- main loop over (b, c) ----------------
        with tc.tile_pool(name="big", bufs=2) as big, \
             tc.tile_pool(name="small", bufs=2) as small, \
             tc.tile_pool(name="pbig", bufs=6, space="PSUM") as pbig, \
             tc.tile_pool(name="psmall", bufs=1, space="PSUM") as psmall:
            def heavy(st):
                """dd/dx, dA, sum-dd/du for a completed iteration (1-deep
                software pipeline: runs while the NEXT iteration's softmax
                is on the DVE/ACT)."""
                b, c, idx, wcol = st["b"], st["c"], st["idx"], st["wcol"]
                Ghs, X0t, g4 = st["Ghs"], st["X0t"], st["g4"]

                dXs = big.tile([P, NCH, E], f32, tag="dx", bufs=3)
                for tl in range(NCH):
                    ddp = pbig.tile([P, E], f32, tag="bigp")
                    for j in range(NCH):
                        nc.tensor.matmul(
                            ddp, Ghs[:, j, P * tl : P * (tl + 1)], X0t[:, j, :],
                            start=(j == 0), stop=(j == NCH - 1))
                    nc.vector.tensor_tensor(
                        out=dXs[:, tl, :], in0=ddp,
                        in1=X0t[:, tl, :].bitcast(f32), op=OP.subtract)
                nc.sync.dma_start(
                    dX_d[b, c, :, :].rearrange("(j p) m -> p j m", p=P), dXs)

                # dA^T = Ghat^T - A0[0,:] (cols), then transpose back
                dAT = big.tile([P, NCH, L], f32, tag="dat")
                for tm in range(NCH):
                    nc.vector.tensor_scalar(
                        out=dAT[:, tm, :], in0=Ghs[:, tm, :].bitcast(f32),
                        scalar1=A0r0s[:, idx * NCH + tm : idx * NCH + tm + 1],
                        scalar2=None, op0=OP.subtract)
                dAs = big.tile([P, NCH, L], f32, tag="da", bufs=2)
                for tl in range(NCH):
                    dap = pbig.tile([P, L], f32, tag="bigp")
                    for j in range(NCH):
                        nc.tensor.transpose(
                            dap[:, P * j : P * (j + 1)],
                            dAT[:, j, P * tl : P * (tl + 1)], IDNs)
                    nc.scalar.copy(dAs[:, tl, :], dap)
                nc.sync.dma_start(
                    dA_d[b, c, :, :].rearrange("(j p) m -> p j m", p=P), dAs)

                # du = w * ((g @ x0)/L - u0)
                sddp = psmall.tile([1, E], f32, tag="vb")
                for t in range(NCH):
                    nc.tensor.matmul(sddp, g4[:, t : t + 1], X0t[:, t, :],
                                     start=(t == 0), stop=(t == NCH - 1))
                u0w = small.tile([1, E], f32, tag="u0w")
                nc.vector.tensor_scalar_mul(
                    u0w, u0rows[0:1, idx, :], wrow[0:1, wcol : wcol + 1])
                durow = small.tile([1, E], f32, tag="du")
                nc.vector.scalar_tensor_tensor(
                    out=durow, in0=sddp,
                    scalar=w512row[0:1, wcol : wcol + 1], in1=u0w,
                    op0=OP.mult, op1=OP.subtract)
                nc.sync.dma_start(dU_d[b, c, :, :], durow)

            pairs = [(rep, b, c) for rep in range(reps)
                     for b in range(BPC) for c in range(C)]
            loaded = {0: (A0t0, X0t0)}
            pending = None
            for k, (rep, b, c) in enumerate(pairs):
                idx = b * C + c
                wcol = BPC * c + b

                A0t, X0t = loaded.pop(k)
                if k + 1 < len(pairs):
                    _, nb, ncc = pairs[k + 1]
                    A0n = big.tile([P, NCH, L], f32r, tag="a0", bufs=3)
                    nc.sync.dma_start(
                        A0n, A0_d[nb, ncc, :, :].rearrange("(j p) m -> p j m", p=P))
                    X0n = big.tile([P, NCH, E], f32r, tag="x0", bufs=3)
                    nc.sync.dma_start(
                        X0n, X0_d[nb, ncc, :, :].rearrange("(j p) m -> p j m", p=P))
                    loaded[k + 1] = (A0n, X0n)

                # scores (DVE) start immediately; Ghat matmuls fill the PE
                vb = psmall.tile([P, E], f32, tag="vb")
                nc.tensor.matmul(vb, EB4s[:, b, :], Vall[:, c, :],
                                 start=True, stop=True)
                if k == 0:
                    Ghs, A511 = Ghs0, A5110
                else:
                    Ghs = big.tile([P, NCH, L], f32r, tag="gh")
                    A511 = small.tile([P, NCH], f32, tag="a511")
                    emit_ghat(A0t, Ghs, A511, pbig)

                S4 = small.tile([P, NCH], f32, tag="s4")
                scr = small.tile([P, E], f32, tag="scr")
                for t in range(NCH):
                    nc.vector.scalar_tensor_tensor(
                        out=scr, in0=X0t[:, t, :].bitcast(f32), scalar=1.0,
                        in1=vb, op0=OP.mult, op1=OP.mult,
                        accum_out=S4[:, t : t + 1])
                srow = psmall.tile([1, L], f32, tag="srow")
                for t in range(NCH):
                    nc.tensor.transpose(
                        srow[0:1, P * t : P * (t + 1)], S4[:, t : t + 1], IDNs)
                negmx = small.tile([1, 1], f32, tag="negmx")
                nc.vector.reduce_max(negmx, srow, axis=AX, negate=True)
                erow = small.tile([1, L], f32, tag="erow")
                ssum = small.tile([1, 1], f32, tag="ssum")
                nc.scalar.activation(erow, srow, AF.Exp, bias=negmx, scale=1.0,
                                     accum_out=ssum)
                rcp = small.tile([1, 1], f32, tag="rcp")
                nc.vector.reciprocal(rcp, ssum)
                prow = small.tile([1, L], f32, tag="prow")
                nc.vector.tensor_scalar_mul(prow, erow, rcp)

                # previous iteration's big matmul block runs here, hiding
                # this iteration's softmax latency
                if pending is not None:
                    heavy(pending)

                pcolp = psmall.tile([P, NCH], f32, tag="srow")
                for t in range(NCH):
                    nc.tensor.transpose(
                        pcolp[:, t : t + 1], prow[0:1, P * t : P * (t + 1)],
                        IDNs[0:1, 0:1])
                P4s = small.tile([P, NCH], f32, tag="p4")
                nc.scalar.copy(P4s, pcolp)
                for tm in range(NCH):
                    nc.scalar.copy(Ghs[:, tm, L - 1 : L], P4s[:, tm : tm + 1])

                # g = A0[511,:] - A0[0,:] + p  (as columns per m-chunk)
                tmp4 = small.tile([P, NCH], f32, tag="tmp4")
                nc.vector.tensor_tensor(
                    out=tmp4, in0=A511,
                    in1=A0r0s[:, idx * NCH : (idx + 1) * NCH], op=OP.subtract)
                g4 = small.tile([P, NCH], f32r, tag="g4")
                nc.vector.tensor_tensor(out=g4, in0=tmp4, in1=P4s, op=OP.add)

                pending = dict(b=b, c=c, idx=idx, wcol=wcol,
                               Ghs=Ghs, X0t=X0t, g4=g4)
            heavy(pending)

    nsplit = _split_excess_waits(nc)
    if nsplit:
        print(f"[kernel] split {nsplit} over-cap sem-wait groups onto NoOps")
    return nc


def _constants():
    ZT = np.zeros((L, L), np.float32)
    li = np.arange(L - 1)
    ZT[li + 1, li] = 1.0
    ZT[li, li] = -1.0
    ZT[L - 1, L - 1] = 1.0  # extracts A0[511,:] into Ghat psum col 511

    IDN = np.eye(P, dtype=np.float32)
    EB4 = np.zeros((BPC, BPC, P), np.float32)
    for b in range(BPC):
        EB4[b, b, :] = 1.0
    return ZT, IDN, EB4


def kernel(A0, x0, u0, t0, t1,
           Q_weight, Q_bias, V_weight, V_bias,
           K_weight, K_bias, T_weight, T_bias):
    from concourse.bass_utils import run_bass_kernel_spmd

    A0 = np.asarray(A0, np.float32)
    x0 = np.asarray(x0, np.float32)
    u0 = np.asarray(u0, np.float32)
    t0 = np.asarray(t0, np.float32)
    t1 = np.asarray(t1, np.float32)

    if "nc" not in _cache:
        _cache["nc"] = _build_program()
    nc = _cache["nc"]

    ZT, IDN, EB4 = _constants()
    shared = {
        "TW": np.asarray(T_weight, np.float32),
        "QW": np.asarray(Q_weight, np.float32),
        "KW": np.asarray(K_weight, np.float32),
        "VWT": np.ascontiguousarray(np.asarray(V_weight, np.float32).transpose(0, 2, 1)),
        "QB": np.asarray(Q_bias, np.float32),
        "KB": np.asarray(K_bias, np.float32),
        "ZT": ZT, "IDN": IDN, "EB4": EB4,
    }

    in_maps = []
    for i in range(NCORES):
        sl = slice(BPC * i, BPC * (i + 1))
        A0s, x0s, u0s = A0[sl], x0[sl], u0[sl]
        m = dict(shared)
        m["A0"] = A0s
        m["x0"] = x0s
        m["t0T"] = np.ascontiguousarray(t0[sl].T)
        m["t1T"] = np.ascontiguousarray(t1[sl].T)
        m["u0T"] = np.ascontiguousarray(u0s.transpose(1, 2, 0))
        m["xlastT"] = np.ascontiguousarray(x0s[:, :, L - 1, :].transpose(1, 2, 0))
        m["u0rows"] = np.ascontiguousarray(u0s.reshape(1, BPC * C, E))
        # A0r0s[p, ((b*C+c)*NCH)+t] = A0[b, c, 0, 128*t+p]
        m["A0r0"] = np.ascontiguousarray(
            A0s[:, :, 0, :].reshape(BPC, C, NCH, P).transpose(3, 0, 1, 2)
            .reshape(P, BPC * C * NCH))
        in_maps.append(m)

    try:
        res = run_bass_kernel_spmd(nc, in_maps, core_ids=list(range(NCORES)))
    except Exception:
        # transient NRT_EXEC_UNIT_UNRECOVERABLE has been observed on the first
        # execution after a fresh load; a retry on the cleared device works
        import time as _time
        _time.sleep(2.0)
        res = run_bass_kernel_spmd(nc, in_maps, core_ids=list(range(NCORES)))

    dA = np.concatenate([r["dA"] for r in res.results], axis=0)
    dX = np.concatenate([r["dX"] for r in res.results], axis=0)
    dU = np.concatenate([r["dU"] for r in res.results], axis=0).reshape(B, C, E)
    return dA, dX, dU
